# revision 6
# baseline (speedup 1.0000x reference)
"""Trainium2 Bass kernel for iterated VQ codebook clustering (nn_Net_34900904247300).

reference:
    for r in 3 iterations:
        sim = (x @ W.T) / ||W_v||        # [B,T,1000]
        idx = argmax_v sim               # [B,T]
        a = W[idx]                       # gather
        a = softmax(a*x, -1) * a         # fused gating
        x = x - a
        anchors.append(a)
    return stack(anchors, 1)             # [B,3,T,D]

Sharding: data-parallel over batch (B=16 over 8 cores, 2 each); codebook
replicated. Each core processes 4096 tokens in 32 tiles of 128 partitions.

v4 structure: tile-major with a persistent per-tile PSUM similarity built
once per tile and updated across iterations by a cheap delta matmul
(sim_{r+1} = sim_r - aoutT @ wn.T; |aout| ~ |x|/30 so one fp16 term is
enough - verified offline: 3 bad rows of 98304, rel err 4.7e-3).

All matmul operands are fp16 (11-bit mantissa = f32r-class precision) so
the PE uses the fast 2-byte weight-load path, and all transposes go through
the DMA xbar instead of the PE.  The similarity lives in a x2048-scaled
domain (argmax is scale-invariant): wh16=fp16(2048*wn), wl16 its fp16
residual, whu16=wh16/2048 pairs with the x2048-scaled x-residual term.
sim1 = xh16*wh16 + xh16*wl16 + xl16*whu16 (exact: 0 flips).

The gating computes aout NEGATED (scalar=-1/sum(e)) so the residual update
x += (-aout) is a free CCE-add DMA and the delta matmul needs no negated
codebook; the host flips the sign during the final reshape.

Tiles are software-pipelined 4 deep (7 stages, skew 2); PSUM holds exactly
4 live similarity tiles (8 banks).
"""
import numpy as np

import concourse.bass as bass
import concourse.bacc as bacc
import concourse.mybir as mybir
import concourse.tile as tile
from concourse.bass_utils import run_bass_kernel_spmd
from concourse.masks import make_identity

P = 128          # partitions / tokens per tile
D = 512          # feature dim
V = 1000         # codebook size
DK = D // P      # 4 contraction chunks
VC = 8           # codebook row chunks (7 full + 104)
N_ITER = 3
N_CORES = 8
TOK = 4096       # tokens per core
NT = TOK // P    # 32 token tiles per core
F32 = mybir.dt.float32
F16 = mybir.dt.float16
AF = mybir.ActivationFunctionType
ALU = mybir.AluOpType

# v-halves aligned to PSUM banks (512 f32 = 1 bank)
V_SPLITS = [(0, 512), (512, V - 512)]

N_STAGES = 7     # 0=prep+sim1, then (argmax, gate+delta) x 3 iters
SKEW = 2         # slots between consecutive tiles -> 4 tiles in flight


def _build():
    nc = bacc.Bacc("TRN2", target_bir_lowering=False, debug=False,
                   num_devices=N_CORES)
    x_d = nc.dram_tensor("x", [TOK, D], F32, kind="ExternalInput")
    w_d = nc.dram_tensor("w", [V, D], F32, kind="ExternalInput")
    out_d = nc.dram_tensor("out", [N_ITER, TOK, D], F32, kind="ExternalOutput")

    with tile.TileContext(nc) as tc:
        with (
            tc.tile_pool(name="const", bufs=1) as const,
            tc.tile_pool(name="wconst", bufs=1) as wconst,
            tc.tile_pool(name="xs", bufs=1) as xs_pool,
            tc.tile_pool(name="xq", bufs=3) as xq,
            tc.tile_pool(name="work", bufs=4) as work,
            tc.tile_pool(name="small", bufs=8) as small,
            tc.tile_pool(name="ps_s", bufs=4, space="PSUM") as ps_s,
        ):
            ident = const.tile([P, P], F32)
            make_identity(nc, ident)

            # ---------- preprocessing: normalized transposed fp16 codebook ----
            wh16 = wconst.tile([P, DK, V], F16, tag="wh16")    # fp16(2048*wnT)
            wl16 = wconst.tile([P, DK, V], F16, tag="wl16")    # fp16 residual
            whu16 = wconst.tile([P, DK, V], F16, tag="whu16")  # wh16 / 2048
            with tc.tile_pool(name="wprep", bufs=1) as wprep:
                w_vp = wprep.tile([P, VC, D], F32, tag="wvp")
                nc.vector.memset(w_vp[:], 1.0)
                for c in range(VC):
                    vlen = V - 7 * P if c == 7 else P
                    nc.sync.dma_start(out=w_vp[:vlen, c, :],
                                      in_=w_d[c * P : c * P + vlen, :])
                norms2 = small.tile([P, VC], F32, tag="n2")
                sq = wprep.tile([P, D], F32, tag="sq")
                for c in range(VC):
                    nc.vector.tensor_mul(sq[:], w_vp[:, c, :], w_vp[:, c, :])
                    nc.vector.reduce_sum(norms2[:, c : c + 1], sq[:],
                                         axis=mybir.AxisListType.X)
                norms = small.tile([P, VC], F32, tag="nrm")
                nc.scalar.sqrt(norms[:], norms2[:])
                inv = small.tile([P, VC], F32, tag="inv")
                nc.vector.reciprocal(inv[:], norms[:])
                inv2k = small.tile([P, VC], F32, tag="inv2k")
                nc.vector.tensor_scalar_mul(inv2k[:], inv[:], 2048.0)
                wn_vp = wprep.tile([P, VC, D], F32, tag="wnvp")
                for c in range(VC):
                    nc.vector.tensor_scalar_mul(wn_vp[:, c, :], w_vp[:, c, :],
                                                inv2k[:, c : c + 1])
                # transpose -> [d_part, dk, v] (x2048-scaled wnT, f32)
                wnT2k = wprep.tile([P, DK, V], F32, tag="wnT2k")
                for c in range(VC):
                    vlen = V - 7 * P if c == 7 else P
                    for k in range(DK):
                        # borrow a psim-tag PSUM buffer for the transpose
                        pt = ps_s.tile([P, V], F32, tag="psim")
                        nc.tensor.transpose(pt[:, :vlen],
                                            wn_vp[:vlen, c, k * P : (k + 1) * P],
                                            ident[:vlen, :vlen])
                        nc.scalar.copy(wnT2k[:, k, c * P : c * P + vlen],
                                       pt[:, :vlen])
                nc.scalar.copy(wh16[:], wnT2k[:])
                nc.vector.tensor_sub(wl16[:], wnT2k[:], wh16[:])
                nc.scalar.activation(whu16[:], wh16[:], AF.Copy,
                                     scale=1.0 / 2048.0)

            # ---------- persistent x tiles ----------
            xs = []
            for ti in range(NT):
                xst = xs_pool.tile([P, D], F32, tag=f"xs{ti}")
                nc.sync.dma_start(out=xst[:], in_=x_d[ti * P : (ti + 1) * P, :])
                xs.append(xst)

            st = [dict() for _ in range(NT)]

            def stage_prep(ti):
                """fp16-split x tile via DMA transposes; 3-term sim1 in PSUM"""
                xh16 = xq.tile([P, D], F16, tag="xh16")
                nc.scalar.copy(xh16[:], xs[ti][:])
                xlf = xq.tile([P, D], F32, tag="xlf")
                nc.vector.tensor_sub(xlf[:], xs[ti][:], xh16[:])
                xl16 = xq.tile([P, D], F16, tag="xl16")
                nc.scalar.activation(xl16[:], xlf[:], AF.Copy, scale=2048.0)
                xTh = xq.tile([P, DK, P], F16, tag="xTh")
                nc.sync.dma_start_transpose(xTh[:], xh16[:])
                xTl = xq.tile([P, DK, P], F16, tag="xTl")
                nc.sync.dma_start_transpose(xTl[:], xl16[:])
                psim = ps_s.tile([P, V], F32, tag="psim")
                # start/stop must be tracked PER PSUM BANK: a group whose
                # flags only bracket the whole 2-bank span miscomputes the
                # second bank on the 2-byte weight path.
                for k in range(DK):
                    for rt in (wh16, wl16):
                        for n0, n1 in V_SPLITS:
                            nc.tensor.matmul(
                                psim[:, n0 : n0 + n1],
                                lhsT=xTh[:, k, :],
                                rhs=rt[:, k, n0 : n0 + n1],
                                start=(k == 0 and rt is wh16),
                                stop=False,
                            )
                for k in range(DK):
                    for n0, n1 in V_SPLITS:
                        nc.tensor.matmul(
                            psim[:, n0 : n0 + n1],
                            lhsT=xTl[:, k, :],
                            rhs=whu16[:, k, n0 : n0 + n1],
                            start=False,
                            stop=(k == DK - 1),
                        )
                st[ti]["psim"] = psim

            def stage_argmax(ti, r):
                psim = st[ti]["psim"]
                m8 = small.tile([P, 8], F32, tag="m8")
                nc.vector.max(out=m8[:], in_=psim[:])
                idx8 = small.tile([P, 8], mybir.dt.uint32, tag="idx8")
                nc.vector.max_index(idx8[:], m8[:], psim[:])
                ag = work.tile([P, D], F32, tag="ag")
                nc.gpsimd.indirect_dma_start(
                    out=ag[:], out_offset=None, in_=w_d[:],
                    in_offset=bass.IndirectOffsetOnAxis(ap=idx8[:, :1], axis=0),
                )
                st[ti]["ag"] = ag

            def stage_gate(ti, r):
                ag = st[ti].pop("ag")
                g = work.tile([P, D], F32, tag="g")
                # g = a*x; |g| <= ~25 so exp cannot overflow in f32
                if ti % 2 == 0:
                    nc.gpsimd.tensor_mul(g[:], ag[:], xs[ti][:])
                else:
                    nc.vector.tensor_mul(g[:], ag[:], xs[ti][:])
                e = work.tile([P, D], F32, tag="e")
                ssum = small.tile([P, 1], F32, tag="ssum")
                nc.scalar.activation(e[:], g[:], AF.Exp, accum_out=ssum[:])
                nssum = small.tile([P, 1], F32, tag="nssum")
                nc.vector.tensor_scalar_mul(nssum[:], ssum[:], -1.0)
                nrinv = small.tile([P, 1], F32, tag="nrinv")
                nc.vector.reciprocal(nrinv[:], nssum[:])       # = -1/sum(e)
                t = work.tile([P, D], F32, tag="t")
                nc.scalar.activation(t[:], e[:], AF.Copy, scale=nrinv[:, :1])
                aoutn = work.tile([P, D], F32, tag="aoutn")    # = -aout
                if ti % 2 == 0:
                    nc.vector.tensor_mul(aoutn[:], t[:], ag[:])
                else:
                    nc.gpsimd.tensor_mul(aoutn[:], t[:], ag[:])
                nc.sync.dma_start(out=out_d[r, ti * P : (ti + 1) * P, :],
                                  in_=aoutn[:])
                if r < N_ITER - 1:
                    # residual x += (-aout) as a CCE-add DMA (no engine time)
                    nc.gpsimd.dma_start(out=xs[ti][:], in_=aoutn[:],
                                        accum_op=ALU.add)
                    # delta: sim += fp16(-aout)T @ wh16  (== sim - aout@wnT)
                    psim = st[ti]["psim"]
                    a16 = xq.tile([P, D], F16, tag="a16")
                    nc.scalar.copy(a16[:], aoutn[:])
                    aTt = xq.tile([P, DK, P], F16, tag="aTt")
                    nc.sync.dma_start_transpose(aTt[:], a16[:])
                    for k in range(DK):
                        for n0, n1 in V_SPLITS:
                            nc.tensor.matmul(
                                psim[:, n0 : n0 + n1],
                                lhsT=aTt[:, k, :],
                                rhs=wh16[:, k, n0 : n0 + n1],
                                start=False,
                                stop=(k == DK - 1),
                                skip_group_check=True,
                            )

            def emit_stage(ti, s):
                if s == 0:
                    stage_prep(ti)
                else:
                    r, sub = divmod(s - 1, 2)
                    if sub == 0:
                        stage_argmax(ti, r)
                    else:
                        stage_gate(ti, r)

            total_slots = (NT - 1) * SKEW + N_STAGES
            for slot in range(total_slots):
                for ti in range(NT - 1, -1, -1):
                    s = slot - ti * SKEW
                    if 0 <= s < N_STAGES:
                        emit_stage(ti, s)

    nc.compile()
    return nc


_NC = None


def _get_nc():
    global _NC
    if _NC is None:
        _NC = _build()
    return _NC


def kernel(x: np.ndarray, embed_weight: np.ndarray) -> np.ndarray:
    x = np.ascontiguousarray(np.asarray(x, dtype=np.float32))
    w = np.ascontiguousarray(np.asarray(embed_weight, dtype=np.float32))
    B, T, Dd = x.shape
    assert (B, T, Dd) == (16, 2048, 512) and w.shape == (V, D)
    nc = _get_nc()
    xs = x.reshape(N_CORES, TOK, D)
    in_maps = [{"x": xs[i], "w": w} for i in range(N_CORES)]
    res = run_bass_kernel_spmd(nc, in_maps, core_ids=list(range(N_CORES)))
    outs = np.stack([res.results[i]["out"] for i in range(N_CORES)])
    # device stores -aout; negate while unsharding
    # [8, 3, 4096, 512] -> [8, 3, 2, 2048, 512] -> [16, 3, 2048, 512]
    out = -outs.reshape(N_CORES, N_ITER, 2, T, D).transpose(0, 2, 1, 3, 4)
    return np.ascontiguousarray(out.reshape(B, N_ITER, T, D))
